# revision 20
# baseline (speedup 1.0000x reference)
"""MultiHeadPool Trainium2 kernel (bf16 dual-layout, host-normalized).

Per-core computation (batch b of 8, one per NeuronCore):
  X = others[b]          (N=64, T=512, D=128)
  L = X . qT * scale     contraction over d   -> (T, H, N) logits
  W = softmax_n(L)
  ctx = W . X            contraction over n   -> (T, H, D)

v3 design -- dual-send layouts (DMA-transpose XBAR measured 33x the cost
model on this backend: it lowers to a descriptor-per-tile-row scatter, so
on-chip transpose is not viable; the HBM dual-send is the cheapest source
of both layouts). Improvements vs the original baseline:
  - mm2 in transposed-output form: stationary = X-pair [jn, d] (from the
    xjn stream), moving = E [jn, 14] -> out ctx^T [d, (j h)] in PSUM.
    Output is fully dense (no 32-row padding), 0.92 MB vs 2.1 MB, and PE
    time per pair drops ~2x (14 moving cols instead of 129).
  - Softmax denominators via ones-row matmuls (lhsT=ones[128,1], rhs=E
    -> [1, n*224]): batched per 2 chunks mid-stream, single-chunk for the
    last two so the den tail stays short. Normalization is done on the
    HOST (raw bf16 numerators + f32 denominators shipped out). Den
    PSUM->SBUF copies ride the ACT engine as Copy activations, delayed
    into idle scalar windows; the DVE only does the output casts.
  - No SBUF tile reuse for inputs => no anti-deps => ALL input DMA groups
    are pre-issued at kernel start (descriptors queue; the 16 DMA queues
    stream back-to-back). Small first groups shorten the time to the
    first matmul; small last groups shorten the tail.
  - Engine plan: sync = ALL input dispatches + output stores (the shared
    HWDGE descriptor generator takes ~1.2us per dma_start, so the engine
    that computes must not also dispatch: scalar runs only the exps and
    the two small denominator stores). The tensor stream is
    software-pipelined two chunks deep (mm1 of chunk g+2 issued before
    mm2 of chunk g) so the PE never idles waiting for exp and stays out
    of the slow post-gap p-states; a dozen dummy matmuls at start warm
    it out of the cold state. The last chunk is processed in half-chunk
    granularity to overlap its serial tail with the final transfers.

Per t-pair c (rows 64j+n, timesteps t=2c+j):
  mm1: L_c[(jn), h] = xd[:,128c:128c+128].T @ qt      (PSUM, f=7)
  exp: E[(jn), c, j'*7+h] = exp(L) on the j==j' diagonal blocks (bf16)
  mm2: ctx^T_c[d, (j h)] = X_c[jn, d].T @ E_c[jn, 14] (PSUM)
  den: ones.T @ E_slots -> [1, n, 16*14] (2-chunk batches; singles at end)
"""

import os
import sys

for p in ("/opt/trn_rl_repo", "/root/.axon_site/_ro/trn_rl_repo"):
    if p not in sys.path:
        sys.path.append(p)

from contextlib import ExitStack

import numpy as np
import ml_dtypes

import concourse.bacc as bacc
import concourse.bass as bass
import concourse.tile as tile
from concourse import mybir
from concourse.bass_utils import run_bass_kernel_spmd

B, N, T, D, H = 8, 64, 512, 128, 7
CH = int(os.environ.get("K_CH", "8"))  # t-pairs per chunk
NG = (T // 2) // CH   # 16 chunks per batch
E2 = 2 * H            # 14 data cols (j-blocked)
F32 = mybir.dt.float32
BF16 = mybir.dt.bfloat16
BF16_NP = ml_dtypes.bfloat16

_CACHE = {}


def _sched(env, default):
    return [tuple(int(x) for x in t.split(":"))
            for t in os.environ.get(env, default).split(",")]


def _body(ctx, tc, xd, xjn, qt, ob, dob):
    nc = tc.nc
    NSLOT = int(os.environ.get("K_NSLOT", "4"))
    if NG == 32:
        d_in = "0:1,1:1,2:2,4:4,8:8,16:8,24:4,28:2,30:1,31:1"
        d_out = "0:8,8:8,16:8,24:4,28:2,30:1,31:1"
    else:
        d_in = "0:1,1:1,2:2,4:4,8:4,12:2,14:1,15:1"
        d_out = "0:4,4:4,8:4,12:2,14:1,15:1"
    XD_G = _sched("K_XD", d_in)
    XJ_G = _sched("K_XJ", d_in)
    OUT_G = _sched("K_OUT", d_out)
    for gs in (XD_G, XJ_G, OUT_G):
        cover = sorted(c for s, l in gs for c in range(s, s + l))
        assert cover == list(range(NG)), cover

    singles = ctx.enter_context(tc.tile_pool(name="singles", bufs=1))
    ltp = ctx.enter_context(tc.tile_pool(name="ltp", bufs=1, space="PSUM"))
    ctxp = ctx.enter_context(tc.tile_pool(
        name="ctxp", bufs=int(os.environ.get("K_CTXP", "4")), space="PSUM"))
    denp = ctx.enter_context(tc.tile_pool(
        name="denp", bufs=int(os.environ.get("K_DENP", "2")), space="PSUM"))
    stg = ctx.enter_context(tc.tile_pool(name="stg", bufs=3))

    qt_sb = singles.tile([D, H], BF16)
    xd_sb = singles.tile([128, NG, CH * 128], BF16)
    xjn_sb = singles.tile([128, NG, CH, D], BF16)
    e_all = singles.tile([128, NSLOT, CH, E2], BF16)
    ones_sb = singles.tile([128, 1], BF16)
    den_sb = singles.tile([1, NG, CH * E2], F32)
    warm = singles.tile([1, 2], F32)

    nc.vector.memset(e_all[:], 0.0)
    nc.vector.memset(ones_sb[:], 1.0)
    nc.vector.memset(warm[:], 0.0)
    # warm the ACT exp table before the first real activation needs it
    nc.scalar.activation(out=warm[:, 1:2], in_=warm[:, 0:1],
                         func=mybir.ActivationFunctionType.Exp)


    # pre-issue every input DMA on the sync engine: no SBUF reuse -> no
    # anti-deps; descriptors generate back-to-back (~1.2us each on the
    # shared HWDGE generator) and stay ahead of the 360 GB/s drain.
    # xd group 0 first (mm1 needs it first), qt next, then interleaved.
    def dma_xd(k):
        s, l = XD_G[k]
        if l == 1 and s == NG - 1:
            # half-chunk granularity for the last chunk: lets mm1/exp/mm2
            # of pairs 0-7 start while pairs 8-15 are still in flight
            nc.sync.dma_start(out=xd_sb[:, s, 0: CH * 64],
                              in_=xd[:, s, 0: CH * 64])
            nc.sync.dma_start(out=xd_sb[:, s, CH * 64: CH * 128],
                              in_=xd[:, s, CH * 64: CH * 128])
        else:
            nc.sync.dma_start(out=xd_sb[:, s: s + l, :], in_=xd[:, s: s + l, :])

    def dma_xj(k):
        s, l = XJ_G[k]
        if l == 1 and s == NG - 1:
            nc.sync.dma_start(out=xjn_sb[:, s, 0: CH // 2],
                              in_=xjn[:, s, 0: CH // 2])
            nc.sync.dma_start(out=xjn_sb[:, s, CH // 2: CH],
                              in_=xjn[:, s, CH // 2: CH])
        else:
            nc.sync.dma_start(out=xjn_sb[:, s: s + l], in_=xjn[:, s: s + l])

    dma_xd(0)
    nc.sync.dma_start(out=qt_sb[:], in_=qt[:])
    dma_xj(0)
    for k in range(1, max(len(XD_G), len(XJ_G))):
        if k < len(XD_G):
            dma_xd(k)
        if k < len(XJ_G):
            dma_xj(k)

    # one persistent L bank with 4 rotating slots (448 fp32 <= 1 bank)
    ltall = ltp.tile([128, 4, CH, H], F32)
    # warm the PE out of its low p-state before the first real mm1
    # (scratch cells in slot 3 of the L bank; mm1(3) overwrites them later)
    for i in range(12):
        c_i, h_i = CH // 2 + (i // 7) % (CH // 2), i % 7
        nc.tensor.matmul(ltall[0:1, 3, c_i, h_i: h_i + 1], lhsT=ones_sb[:],
                         rhs=ones_sb[:], start=True, stop=True)

    out_start = {s: (s, l) for s, l in OUT_G}
    out_end = {s + l - 1: (s, l) for s, l in OUT_G}
    out_rank = {s: i for i, (s, l) in enumerate(OUT_G)}
    state = {}

    def mm1(g):
        ltb = ltall[:, g % 4]
        for i in range(CH):
            nc.tensor.matmul(
                ltb[:, i, :],
                lhsT=xd_sb[:, g, 128 * i: 128 * (i + 1)],
                rhs=qt_sb[:],
                start=True, stop=True,
            )

    mm1(0)
    mm1(1)
    for g in range(NG):
        # software pipeline: issue mm1 two chunks ahead so the PE has
        # runway while the scalar engine runs this chunk's exp
        if g + 2 < NG:
            mm1(g + 2)
        ltb = ltall[:, g % 4]

        e_g = e_all[:, g % NSLOT]
        h_ch = CH // 2
        exp_splits = ([(0, h_ch), (h_ch, CH)] if g == NG - 1 else [(0, CH)])
        for c0, c1 in exp_splits:
            nc.scalar.activation(
                out=e_g[0:64, c0:c1, 0:H], in_=ltb[0:64, c0:c1],
                func=mybir.ActivationFunctionType.Exp,
            )
            nc.scalar.activation(
                out=e_g[64:128, c0:c1, H:E2], in_=ltb[64:128, c0:c1],
                func=mybir.ActivationFunctionType.Exp,
            )

        # mm2: ctx^T per pair; denominators for the whole chunk in one mm
        ctxb = ctxp.tile([128, CH, E2], F32)
        for c in range(CH):
            nc.tensor.matmul(
                ctxb[:, c, :],
                lhsT=xjn_sb[:, g, c, :],
                rhs=e_g[:, c, :],
                start=True, stop=True,
            )
        if g % 2 == 1 and g < NG - 2:
            denb = denp.tile([1, 2, CH * E2], F32)
            s0 = (g - 1) % NSLOT
            nc.tensor.matmul(
                denb[:],
                lhsT=ones_sb[:],
                rhs=e_all[:, s0: s0 + 2],
                start=True, stop=True,
            )
            state["denb%d" % g] = denb
        elif g >= NG - 2:
            # single-chunk dens for the last two chunks: keeps the final
            # den matmul + copy small, so the den path does not outlast
            # the main output-store path at the tail
            denb = denp.tile([1, 2, CH * E2], F32)
            nc.tensor.matmul(
                denb[:, 0],
                lhsT=ones_sb[:],
                rhs=e_g[:],
                start=True, stop=True,
            )
            state["denb%d" % g] = denb

        if g in out_start:
            og0, ogl = out_start[g]
            state["og0"] = og0
            stc_new = stg.tile([128, ogl, CH, E2], BF16)
            state["stc_fg"] = stc_new
        stc_fg = state["stc_fg"]
        if g == NG - 1:
            nc.vector.tensor_copy(
                stc_fg[:, g - state["og0"], 0: h_ch], ctxb[:, 0: h_ch])
            nc.sync.dma_start(out=ob[:, g, 0: h_ch],
                              in_=stc_fg[:, g - state["og0"], 0: h_ch])
            nc.vector.tensor_copy(
                stc_fg[:, g - state["og0"], h_ch:], ctxb[:, h_ch:])
        else:
            nc.vector.tensor_copy(stc_fg[:, g - state["og0"]], ctxb[:])
        if g % 2 == 1 and 3 <= g <= NG - 3:
            # den copies ride the ACT engine (Copy activation), delayed by
            # one chunk pair so they sit in idle scalar windows instead of
            # gating the next exp (on DVE they would gate the output casts;
            # the Pool engine cannot read PSUM)
            nc.scalar.activation(
                out=den_sb[:, g - 3: g - 1, :], in_=state["denb%d" % (g - 2)][:],
                func=mybir.ActivationFunctionType.Copy,
            )
        if g == NG - 2:
            nc.scalar.activation(
                out=den_sb[:, NG - 4: NG - 2, :],
                in_=state["denb%d" % (NG - 3)][:],
                func=mybir.ActivationFunctionType.Copy,
            )
        if g == NG - 1:
            nc.scalar.activation(
                out=den_sb[:, NG - 2: NG - 1, :],
                in_=state["denb%d" % (NG - 2)][:, 0],
                func=mybir.ActivationFunctionType.Copy,
            )

        if g in out_end:
            og0, ogl = out_end[g]
            if g == NG - 1 and ogl == 1:
                nc.sync.dma_start(out=ob[:, g, h_ch:],
                                  in_=stc_fg[:, g - state["og0"], h_ch:])
            else:
                nc.sync.dma_start(out=ob[:, og0: og0 + ogl], in_=stc_fg[:])
        if g == NG - 3:
            # bulk denominator store on sync (idle-waiting by then);
            # gpsimd now carries nothing at all
            nc.sync.dma_start(out=dob[:, : NG - 4], in_=den_sb[:, : NG - 4])

    # tail: only chunk 15's tiny den copy + the small den store remain
    nc.scalar.activation(
        out=den_sb[:, NG - 1:, :], in_=state["denb%d" % (NG - 1)][:, 0],
        func=mybir.ActivationFunctionType.Copy,
    )
    nc.scalar.dma_start(out=dob[:, NG - 4:], in_=den_sb[:, NG - 4:])


def _build():
    # Bacc (not bare Bass): its compile() runs move_matmul_waits_to_ldweights
    # + generate_event_semaphores, which legalize multi-wait instructions for
    # the TRN2 one-wait-per-instruction constraint.
    nc = bacc.Bacc("TRN2", target_bir_lowering=False, debug=False)
    xd = nc.dram_tensor("xd", [128, NG, CH * 128], BF16, kind="ExternalInput")
    xjn = nc.dram_tensor("xjn", [128, NG, CH, D], BF16, kind="ExternalInput")
    qt = nc.dram_tensor("qt", [D, H], BF16, kind="ExternalInput")
    # raw numerators: [d, chunk, pair, (j h)]; host divides by denominators
    ob = nc.dram_tensor("ob", [128, NG, CH, E2], BF16, kind="ExternalOutput")
    dob = nc.dram_tensor("dob", [1, NG, CH * E2], F32, kind="ExternalOutput")
    with tile.TileContext(nc) as tc:
        with ExitStack() as ctx:
            _body(ctx, tc, xd[:], xjn[:], qt[:], ob[:], dob[:])
    nc.compile()
    return nc


def get_nc():
    if "nc" not in _CACHE:
        _CACHE["nc"] = _build()
    return _CACHE["nc"]


def prep_inputs(others_b):
    """others[b] (N,T,D) -> (xd, xjn) bf16 layouts.

    xd[d, g, 128c'+64j+n] = others[n, t, d] for t = 2(16g+c')+j
    xjn[64j+n, g, c', d]  = others[n, 2(16g+c')+j, d]
    """
    xd = np.ascontiguousarray(
        others_b.transpose(2, 1, 0)                 # d, t, n
    ).reshape(128, NG, CH * 128).astype(BF16_NP)
    v = others_b.reshape(N, NG, CH, 2, D)           # n, g, c, j, d
    xjn = np.empty((128, NG, CH, D), dtype=BF16_NP)
    xjn[0:64] = v[:, :, :, 0, :]
    xjn[64:128] = v[:, :, :, 1, :]
    return xd, xjn


def kernel(ego=None, others=None, queries=None, _trace=False, **_unused):
    others = np.asarray(others, dtype=np.float32)
    queries = np.asarray(queries, dtype=np.float32)
    scale = float(queries.shape[-1]) ** -0.5
    qt_scaled = np.ascontiguousarray(queries.T * scale).astype(BF16_NP)

    nc = get_nc()
    in_maps = []
    for b in range(B):
        xd, xjn = prep_inputs(others[b])
        in_maps.append({"xd": xd, "xjn": xjn, "qt": qt_scaled})
    res = run_bass_kernel_spmd(nc, in_maps, core_ids=list(range(B)), trace=_trace)
    _CACHE["last_results"] = res
    out = np.empty((B, T, H, D), dtype=np.float32)
    for b in range(B):
        out[b] = unpack_output(res.results[b]["ob"], res.results[b]["dob"])
    return out


def unpack_output(ob_raw, den_raw):
    """ob [128, NG, CH, (j h)] bf16 + den [1, NG, CH*(j h)] f32 -> (T, H, D)."""
    num = ob_raw.astype(np.float32).transpose(1, 2, 3, 0)   # g, c, m, d
    num = num.reshape(T // 2, 2, H, D)                      # tp, j, h, d
    den = den_raw.reshape(NG, CH, 2, H).astype(np.float32)
    den = den.reshape(T // 2, 2, H)
    return np.ascontiguousarray(
        (num / den[..., None]).reshape(T, H, D)
    )


# revision 22
# speedup vs baseline: 1.2719x; 1.2719x over previous
"""MultiHeadPool Trainium2 kernel (bf16 dual-layout, host-normalized).

Per-core computation (batch b of 8, one per NeuronCore):
  X = others[b]          (N=64, T=512, D=128)
  L = X . qT * scale     contraction over d   -> (T, H, N) logits
  W = softmax_n(L)
  ctx = W . X            contraction over n   -> (T, H, D)

v3 design -- dual-send layouts (DMA-transpose XBAR measured 33x the cost
model on this backend: it lowers to a descriptor-per-tile-row scatter, so
on-chip transpose is not viable; the HBM dual-send is the cheapest source
of both layouts). Improvements vs the original baseline:
  - mm2 in transposed-output form: stationary = X-pair [jn, d] (from the
    xjn stream), moving = E [jn, 14] -> out ctx^T [d, (j h)] in PSUM.
    Output is fully dense (no 32-row padding), 0.92 MB vs 2.1 MB, and PE
    time per pair drops ~2x (14 moving cols instead of 129).
  - Softmax denominators via ones-row matmuls (lhsT=ones[128,1], rhs=E
    -> [1, n*224]): batched per 2 chunks mid-stream, single-chunk for the
    last two so the den tail stays short. Normalization is done on the
    HOST (raw bf16 numerators + f32 denominators shipped out). Den
    PSUM->SBUF copies ride the ACT engine as Copy activations, delayed
    into idle scalar windows; the DVE only does the output casts.
  - No SBUF tile reuse for inputs => no anti-deps => ALL input DMA groups
    are pre-issued at kernel start (descriptors queue; the 16 DMA queues
    stream back-to-back). Small first groups shorten the time to the
    first matmul; small last groups shorten the tail.
  - Engine plan: sync = ALL input dispatches + output stores (the shared
    HWDGE descriptor generator takes ~1.2us per dma_start, so the engine
    that computes must not also dispatch: scalar runs only the exps and
    the two small denominator stores). The tensor stream is
    software-pipelined two chunks deep (mm1 of chunk g+2 issued before
    mm2 of chunk g) so the PE never idles waiting for exp and stays out
    of the slow post-gap p-states; a dozen dummy matmuls at start warm
    it out of the cold state. The last chunk is processed in half-chunk
    granularity to overlap its serial tail with the final transfers.

Per t-pair c (rows 64j+n, timesteps t=2c+j):
  mm1: L_c[(jn), h] = xd[:,128c:128c+128].T @ qt      (PSUM, f=7)
  exp: E[(jn), c, j'*7+h] = exp(L) on the j==j' diagonal blocks (bf16)
  mm2: ctx^T_c[d, (j h)] = X_c[jn, d].T @ E_c[jn, 14] (PSUM)
  den: ones.T @ E_slots -> [1, n, 16*14] (2-chunk batches; singles at end)
"""

import os
import sys

for p in ("/opt/trn_rl_repo", "/root/.axon_site/_ro/trn_rl_repo"):
    if p not in sys.path:
        sys.path.append(p)

from contextlib import ExitStack

import numpy as np
import ml_dtypes

import concourse.bacc as bacc
import concourse.bass as bass
import concourse.tile as tile
from concourse import mybir
from concourse.bass_utils import run_bass_kernel_spmd

B, N, T, D, H = 8, 64, 512, 128, 7
CH = int(os.environ.get("K_CH", "16"))  # t-pairs per chunk
NG = (T // 2) // CH   # 16 chunks per batch
E2 = 2 * H            # 14 data cols (j-blocked)
F32 = mybir.dt.float32
BF16 = mybir.dt.bfloat16
BF16_NP = ml_dtypes.bfloat16

_CACHE = {}


def _sched(env, default):
    return [tuple(int(x) for x in t.split(":"))
            for t in os.environ.get(env, default).split(",")]


def _body(ctx, tc, xd, xjn, qt, ob, dob):
    nc = tc.nc
    NSLOT = int(os.environ.get("K_NSLOT", "4"))
    if NG == 32:
        d_in = "0:1,1:1,2:2,4:4,8:8,16:8,24:4,28:2,30:1,31:1"
        d_out = "0:8,8:8,16:8,24:4,28:2,30:1,31:1"
    else:
        d_in = "0:1,1:1,2:2,4:4,8:4,12:2,14:1,15:1"
        d_out = "0:4,4:4,8:4,12:2,14:1,15:1"
    XD_G = _sched("K_XD", d_in)
    XJ_G = _sched("K_XJ", d_in)
    OUT_G = _sched("K_OUT", d_out)
    for gs in (XD_G, XJ_G, OUT_G):
        cover = sorted(c for s, l in gs for c in range(s, s + l))
        assert cover == list(range(NG)), cover

    singles = ctx.enter_context(tc.tile_pool(name="singles", bufs=1))
    ltp = ctx.enter_context(tc.tile_pool(name="ltp", bufs=1, space="PSUM"))
    ctxp = ctx.enter_context(tc.tile_pool(
        name="ctxp", bufs=int(os.environ.get("K_CTXP", "4")), space="PSUM"))
    denp = ctx.enter_context(tc.tile_pool(
        name="denp", bufs=int(os.environ.get("K_DENP", "2")), space="PSUM"))
    stg = ctx.enter_context(tc.tile_pool(name="stg", bufs=3))

    qt_sb = singles.tile([D, H], BF16)
    xd_sb = singles.tile([128, NG, CH * 128], BF16)
    xjn_sb = singles.tile([128, NG, CH, D], BF16)
    e_all = singles.tile([128, NSLOT, CH, E2], BF16)
    ones_sb = singles.tile([128, 1], BF16)
    den_sb = singles.tile([1, NG, CH * E2], F32)
    warm = singles.tile([1, 2], F32)

    nc.vector.memset(e_all[:], 0.0)
    nc.vector.memset(ones_sb[:], 1.0)
    nc.vector.memset(warm[:], 0.0)
    # warm the ACT exp table before the first real activation needs it
    nc.scalar.activation(out=warm[:, 1:2], in_=warm[:, 0:1],
                         func=mybir.ActivationFunctionType.Exp)


    # pre-issue every input DMA on the sync engine: no SBUF reuse -> no
    # anti-deps; descriptors generate back-to-back (~1.2us each on the
    # shared HWDGE generator) and stay ahead of the 360 GB/s drain.
    # xd group 0 first (mm1 needs it first), qt next, then interleaved.
    def dma_xd(k):
        s, l = XD_G[k]
        if l == 1 and s == NG - 1:
            # half-chunk granularity for the last chunk: lets mm1/exp/mm2
            # of pairs 0-7 start while pairs 8-15 are still in flight
            nc.sync.dma_start(out=xd_sb[:, s, 0: CH * 64],
                              in_=xd[:, s, 0: CH * 64])
            nc.sync.dma_start(out=xd_sb[:, s, CH * 64: CH * 128],
                              in_=xd[:, s, CH * 64: CH * 128])
        else:
            nc.sync.dma_start(out=xd_sb[:, s: s + l, :], in_=xd[:, s: s + l, :])

    def dma_xj(k):
        s, l = XJ_G[k]
        if l == 1 and s == NG - 1:
            nc.sync.dma_start(out=xjn_sb[:, s, 0: CH // 2],
                              in_=xjn[:, s, 0: CH // 2])
            nc.sync.dma_start(out=xjn_sb[:, s, CH // 2: CH],
                              in_=xjn[:, s, CH // 2: CH])
        else:
            nc.sync.dma_start(out=xjn_sb[:, s: s + l], in_=xjn[:, s: s + l])

    dma_xd(0)
    nc.sync.dma_start(out=qt_sb[:], in_=qt[:])
    dma_xj(0)
    for k in range(1, max(len(XD_G), len(XJ_G))):
        if k < len(XD_G):
            dma_xd(k)
        if k < len(XJ_G):
            dma_xj(k)

    # one persistent L bank with 4 rotating slots (448 fp32 <= 1 bank)
    ltall = ltp.tile([128, 4, CH, H], F32)
    # warm the PE out of its low p-state before the first real mm1
    # (scratch cells in slot 3 of the L bank; mm1(3) overwrites them later)
    for i in range(12):
        c_i, h_i = CH // 2 + (i // 7) % (CH // 2), i % 7
        nc.tensor.matmul(ltall[0:1, 3, c_i, h_i: h_i + 1], lhsT=ones_sb[:],
                         rhs=ones_sb[:], start=True, stop=True)

    out_start = {s: (s, l) for s, l in OUT_G}
    out_end = {s + l - 1: (s, l) for s, l in OUT_G}
    out_rank = {s: i for i, (s, l) in enumerate(OUT_G)}
    state = {}

    def mm1(g):
        ltb = ltall[:, g % 4]
        for i in range(CH):
            nc.tensor.matmul(
                ltb[:, i, :],
                lhsT=xd_sb[:, g, 128 * i: 128 * (i + 1)],
                rhs=qt_sb[:],
                start=True, stop=True,
            )

    mm1(0)
    mm1(1)
    mm1(2)
    for g in range(NG):
        # software pipeline: issue mm1 three chunks ahead so the PE has
        # runway while the scalar engine runs this chunk's exp (safe with
        # the 4-slot L bank: mm1(g+3) only WAR-depends on exp(g-1))
        if g + 3 < NG:
            mm1(g + 3)
        ltb = ltall[:, g % 4]

        e_g = e_all[:, g % NSLOT]
        h_ch = CH // 2
        exp_splits = ([(0, h_ch), (h_ch, CH)] if g == NG - 1 else [(0, CH)])
        for c0, c1 in exp_splits:
            nc.scalar.activation(
                out=e_g[0:64, c0:c1, 0:H], in_=ltb[0:64, c0:c1],
                func=mybir.ActivationFunctionType.Exp,
            )
            nc.scalar.activation(
                out=e_g[64:128, c0:c1, H:E2], in_=ltb[64:128, c0:c1],
                func=mybir.ActivationFunctionType.Exp,
            )

        # mm2: ctx^T per pair; denominators for the whole chunk in one mm
        ctxb = ctxp.tile([128, CH, E2], F32)
        for c in range(CH):
            nc.tensor.matmul(
                ctxb[:, c, :],
                lhsT=xjn_sb[:, g, c, :],
                rhs=e_g[:, c, :],
                start=True, stop=True,
            )
        if g % 2 == 1 and g < NG - 2:
            denb = denp.tile([1, 2, CH * E2], F32)
            s0 = (g - 1) % NSLOT
            nc.tensor.matmul(
                denb[:],
                lhsT=ones_sb[:],
                rhs=e_all[:, s0: s0 + 2],
                start=True, stop=True,
            )
            state["denb%d" % g] = denb
        elif g >= NG - 2:
            # single-chunk dens for the last two chunks: keeps the final
            # den matmul + copy small, so the den path does not outlast
            # the main output-store path at the tail
            denb = denp.tile([1, 2, CH * E2], F32)
            nc.tensor.matmul(
                denb[:, 0],
                lhsT=ones_sb[:],
                rhs=e_g[:],
                start=True, stop=True,
            )
            state["denb%d" % g] = denb

        if g in out_start:
            og0, ogl = out_start[g]
            state["og0"] = og0
            stc_new = stg.tile([128, ogl, CH, E2], BF16)
            state["stc_fg"] = stc_new
        stc_fg = state["stc_fg"]
        if g == NG - 1:
            nc.vector.tensor_copy(
                stc_fg[:, g - state["og0"], 0: h_ch], ctxb[:, 0: h_ch])
            nc.sync.dma_start(out=ob[:, g, 0: h_ch],
                              in_=stc_fg[:, g - state["og0"], 0: h_ch])
            nc.vector.tensor_copy(
                stc_fg[:, g - state["og0"], h_ch:], ctxb[:, h_ch:])
        else:
            nc.vector.tensor_copy(stc_fg[:, g - state["og0"]], ctxb[:])
        if g % 2 == 1 and 3 <= g <= NG - 3:
            # den copies ride the ACT engine (Copy activation), delayed by
            # one chunk pair so they sit in idle scalar windows instead of
            # gating the next exp (on DVE they would gate the output casts;
            # the Pool engine cannot read PSUM)
            nc.scalar.activation(
                out=den_sb[:, g - 3: g - 1, :], in_=state["denb%d" % (g - 2)][:],
                func=mybir.ActivationFunctionType.Copy,
            )
        if g == NG - 2:
            nc.scalar.activation(
                out=den_sb[:, NG - 4: NG - 2, :],
                in_=state["denb%d" % (NG - 3)][:],
                func=mybir.ActivationFunctionType.Copy,
            )
        if g == NG - 1:
            nc.scalar.activation(
                out=den_sb[:, NG - 2: NG - 1, :],
                in_=state["denb%d" % (NG - 2)][:, 0],
                func=mybir.ActivationFunctionType.Copy,
            )

        if g in out_end:
            og0, ogl = out_end[g]
            if g == NG - 1 and ogl == 1:
                nc.sync.dma_start(out=ob[:, g, h_ch:],
                                  in_=stc_fg[:, g - state["og0"], h_ch:])
            else:
                nc.sync.dma_start(out=ob[:, og0: og0 + ogl], in_=stc_fg[:])
        if g == NG - 3:
            # bulk denominator store on sync (idle-waiting by then);
            # gpsimd now carries nothing at all
            nc.sync.dma_start(out=dob[:, : NG - 4], in_=den_sb[:, : NG - 4])

    # tail: only chunk 15's tiny den copy + the small den store remain
    nc.scalar.activation(
        out=den_sb[:, NG - 1:, :], in_=state["denb%d" % (NG - 1)][:, 0],
        func=mybir.ActivationFunctionType.Copy,
    )
    nc.scalar.dma_start(out=dob[:, NG - 4:], in_=den_sb[:, NG - 4:])


def _build():
    # Bacc (not bare Bass): its compile() runs move_matmul_waits_to_ldweights
    # + generate_event_semaphores, which legalize multi-wait instructions for
    # the TRN2 one-wait-per-instruction constraint.
    nc = bacc.Bacc("TRN2", target_bir_lowering=False, debug=False)
    xd = nc.dram_tensor("xd", [128, NG, CH * 128], BF16, kind="ExternalInput")
    xjn = nc.dram_tensor("xjn", [128, NG, CH, D], BF16, kind="ExternalInput")
    qt = nc.dram_tensor("qt", [D, H], BF16, kind="ExternalInput")
    # raw numerators: [d, chunk, pair, (j h)]; host divides by denominators
    ob = nc.dram_tensor("ob", [128, NG, CH, E2], BF16, kind="ExternalOutput")
    dob = nc.dram_tensor("dob", [1, NG, CH * E2], F32, kind="ExternalOutput")
    with tile.TileContext(nc) as tc:
        with ExitStack() as ctx:
            _body(ctx, tc, xd[:], xjn[:], qt[:], ob[:], dob[:])
    nc.compile()
    return nc


def get_nc():
    if "nc" not in _CACHE:
        _CACHE["nc"] = _build()
    return _CACHE["nc"]


def prep_inputs(others_b):
    """others[b] (N,T,D) -> (xd, xjn) bf16 layouts.

    xd[d, g, 128c'+64j+n] = others[n, t, d] for t = 2(16g+c')+j
    xjn[64j+n, g, c', d]  = others[n, 2(16g+c')+j, d]
    """
    xd = np.ascontiguousarray(
        others_b.transpose(2, 1, 0)                 # d, t, n
    ).reshape(128, NG, CH * 128).astype(BF16_NP)
    v = others_b.reshape(N, NG, CH, 2, D)           # n, g, c, j, d
    xjn = np.empty((128, NG, CH, D), dtype=BF16_NP)
    xjn[0:64] = v[:, :, :, 0, :]
    xjn[64:128] = v[:, :, :, 1, :]
    return xd, xjn


def kernel(ego=None, others=None, queries=None, _trace=False, **_unused):
    others = np.asarray(others, dtype=np.float32)
    queries = np.asarray(queries, dtype=np.float32)
    scale = float(queries.shape[-1]) ** -0.5
    qt_scaled = np.ascontiguousarray(queries.T * scale).astype(BF16_NP)

    nc = get_nc()
    in_maps = []
    for b in range(B):
        xd, xjn = prep_inputs(others[b])
        in_maps.append({"xd": xd, "xjn": xjn, "qt": qt_scaled})
    res = run_bass_kernel_spmd(nc, in_maps, core_ids=list(range(B)), trace=_trace)
    _CACHE["last_results"] = res
    out = np.empty((B, T, H, D), dtype=np.float32)
    for b in range(B):
        out[b] = unpack_output(res.results[b]["ob"], res.results[b]["dob"])
    return out


def unpack_output(ob_raw, den_raw):
    """ob [128, NG, CH, (j h)] bf16 + den [1, NG, CH*(j h)] f32 -> (T, H, D)."""
    num = ob_raw.astype(np.float32).transpose(1, 2, 3, 0)   # g, c, m, d
    num = num.reshape(T // 2, 2, H, D)                      # tp, j, h, d
    den = den_raw.reshape(NG, CH, 2, H).astype(np.float32)
    den = den.reshape(T // 2, 2, H)
    return np.ascontiguousarray(
        (num / den[..., None]).reshape(T, H, D)
    )
